# revision 1
# baseline (speedup 1.0000x reference)
"""AAMaxPool2d Trainium2 kernel: 8-core data-parallel (4 images/core).

Pipeline per (n,c) plane (112x112 -> 56x56), all on-chip:
  u  = Tpre^T X Tpre          (2x upsample + 7-tap pre-blur, banded Toeplitz matmuls on PE)
  m  = maxpool5x5_s1_pad1(u)  (separable 5-window max cascades on DVE, bf16 2x mode)
  y  = Tpost^T m Tpost        (7-tap post-blur + 4x downsample, Toeplitz matmuls on PE)
Transposes between axis passes run on PE (identity matmul). PSUM evacuations on ACT/DVE.
"""
import os
import numpy as np
import ml_dtypes

from contextlib import ExitStack
from concourse import bass, mybir
from concourse.bass_utils import run_bass_kernel_spmd
from concourse.tile import TileContext
import bass_rust

F32 = mybir.dt.float32
BF16 = mybir.dt.bfloat16
MAX = mybir.AluOpType.max

N_CORES = 8
N_PER = 4          # images per core
C = 64
H = W = 112
G = 4              # planes (channels) per group
N_GROUPS = int(os.environ.get("KERNEL_GROUPS", N_PER * (C // G)))  # 64
NEG = -3.0e38


def _build_toeplitz(lpf_pre, lpf_post):
    f = np.asarray(lpf_pre, np.float64)
    g = np.asarray(lpf_post, np.float64)
    tpre = np.zeros((H, 2 * H), np.float64)
    for o in range(2 * H):
        for t in range(7):
            j = o + t - 3
            if j >= 0 and j % 2 == 0 and j // 2 < H:
                tpre[j // 2, o] += f[t]
    tpost = np.zeros((222, 56), np.float64)
    for o in range(56):
        for t in range(7):
            i = 4 * o + t - 3
            if 0 <= i < 222:
                tpost[i, o] += g[t]
    return tpre, tpost


def split_multi_waits(nc):
    """This walrus build accepts only ONE semaphore wait per instruction;
    hoist extras onto preceding same-engine NoOps."""
    k = 0
    for fn in nc.m.functions:
        for bb in fn.blocks:
            new = []
            for inst in bb.instructions:
                si = inst.sync_info
                if si is not None and len(si.on_wait) > 1:
                    waits = list(si.on_wait)
                    for wt in waits[:-1]:
                        nop = mybir.InstNoOp(name=f"WS-{k}", ins=[], outs=[])
                        k += 1
                        nop.engine = inst.engine
                        nop.sync_info = bass_rust.SyncInfo(on_wait=[wt], on_update=[])
                        new.append(nop)
                    inst.sync_info = bass_rust.SyncInfo(
                        on_wait=[waits[-1]], on_update=list(si.on_update)
                    )
                new.append(inst)
            bb.instructions = new


def _emit_max5(nc, pool, mpool, psum, parts, tag):
    """5-window max (pad 1) along the innermost free axis of psum [parts, 2, 224]
    -> returns bf16 tile [parts, 2, 222]."""
    v = pool.tile([parts, 2, 224], BF16, tag=f"v{tag}")
    nc.any.tensor_copy(v[:], psum[:, :, 0:224])
    # w[k] = v[k-1] for k in 1..224; w[0]=v[0], w[225]=v[223] (window clamping)
    w = pool.tile([parts, 2, 228], BF16, tag=f"w{tag}")
    nc.any.tensor_copy(w[:, :, 1:225], psum[:, :, 0:224])
    nc.vector.tensor_copy(w[:, :, 0:1], psum[:, :, 0:1])
    nc.vector.tensor_copy(w[:, :, 225:226], psum[:, :, 223:224])
    a = pool.tile([parts, 2, 224], BF16, tag=f"a{tag}")
    nc.vector.tensor_tensor(a[:], v[:], w[:, :, 0:224], MAX)
    b = pool.tile([parts, 2, 222], BF16, tag=f"b{tag}")
    nc.vector.tensor_tensor(b[:], a[:, :, 0:222], a[:, :, 2:224], MAX)
    m = mpool.tile([parts, 2, 222], BF16, tag=f"m{tag}")
    nc.vector.tensor_tensor(m[:], b[:], w[:, :, 4:226], MAX)
    return m


def _build(split=True):
    nc = bass.Bass()
    x = nc.declare_dram_parameter("x", [N_PER, C, H, W], F32, isOutput=False)
    tpre_d = nc.declare_dram_parameter("tpre", [112, 224], BF16, isOutput=False)
    tpa_d = nc.declare_dram_parameter("tpost_a", [112, 56], BF16, isOutput=False)
    tpb_d = nc.declare_dram_parameter("tpost_b", [110, 56], BF16, isOutput=False)
    id_d = nc.declare_dram_parameter("identb", [128, 128], BF16, isOutput=False)
    out = nc.declare_dram_parameter("out", [N_PER, C, 56, 56], F32, isOutput=True)

    with TileContext(nc) as tc, ExitStack() as ctx:
        wp = ctx.enter_context(tc.tile_pool(name="weights", bufs=1))
        tpre = wp.tile([112, 224], BF16, tag="tpre")
        tpa = wp.tile([112, 56], BF16, tag="tpa")
        tpb = wp.tile([110, 56], BF16, tag="tpb")
        ident = wp.tile([128, 128], BF16, tag="ident")
        nc.sync.dma_start(tpre[:], tpre_d[:])
        nc.sync.dma_start(tpa[:], tpa_d[:])
        nc.sync.dma_start(tpb[:], tpb_d[:])
        nc.sync.dma_start(ident[:], id_d[:])

        ps = ctx.enter_context(tc.tile_pool(name="ps", bufs=8, space="PSUM"))
        sb = ctx.enter_context(tc.tile_pool(name="sb", bufs=6))
        sbm = ctx.enter_context(tc.tile_pool(name="sbm", bufs=6))

        for gi in range(N_GROUPS):
            n, cg = divmod(gi, C // G)
            c0 = cg * G
            # S0: load 4 planes [112H p, 4c, 112W]
            xf = sb.tile([112, G, 112], F32, tag="xf")
            nc.sync.dma_start(xf[:], x[n, c0 : c0 + G, :, :].rearrange("c h w -> h c w"))
            xb = sb.tile([112, G, 112], BF16, tag="xb")
            nc.vector.tensor_copy(xb[:], xf[:])

            # S2: pre H-conv  u[o,(c,w)] = sum_h tpre[h,o] x[h,c,w]
            u_sb = []
            for k in range(2):
                pu = ps.tile([112, G, 112], F32, tag="ps")
                nc.tensor.matmul(pu[:], tpre[:, 112 * k : 112 * (k + 1)], xb[:], start=True, stop=True)
                us = sb.tile([112, G, 112], BF16, tag=f"us{k}")
                nc.any.tensor_copy(us[:], pu[:])
                u_sb.append(us)

            # S4: transpose planes -> uT [112W, 4c, 224H']
            uT = sb.tile([112, G, 224], BF16, tag="uT")
            for k in range(2):
                pt = ps.tile([112, G, 112], BF16, tag="ps")
                for p in range(G):
                    nc.tensor.transpose(pt[:, p, :], u_sb[k][:, p, :], ident[0:112, 0:112])
                nc.any.tensor_copy(uT[:, :, 112 * k : 112 * (k + 1)], pt[:])

            # S6: pre W-conv; S7: H'-max  -> hm tiles [112 W'm, 2, 222]
            hm = {}
            for m in range(2):
                for j in range(2):
                    pv = ps.tile([112, 2, 224], F32, tag="ps")
                    nc.tensor.matmul(
                        pv[:], tpre[:, 112 * m : 112 * (m + 1)],
                        uT[:, 2 * j : 2 * j + 2, :], start=True, stop=True,
                    )
                    hm[(m, j)] = _emit_max5(nc, sb, sbm, pv, 112, "h")

            # S8: transpose -> psum_z [ls, 2planes, 224W']; S9: W'-max -> zm [ls, 2, 222]
            zm = {}
            for s in range(2):
                ls = 112 if s == 0 else 110
                for j in range(2):
                    pz = ps.tile([ls, 2, 224], BF16, tag="ps")
                    for p in range(2):
                        for m in range(2):
                            nc.tensor.transpose(
                                pz[:, p, 112 * m : 112 * (m + 1)],
                                hm[(m, j)][:, p, 112 * s : 112 * s + ls],
                                ident[0:112, 0:112],
                            )
                    zm[(s, j)] = _emit_max5(nc, sb, sbm, pz, ls, "w")

            # S10: post H-conv (contract H'' over 2 partition chunks, accumulate)
            o_sb = []
            for j in range(2):
                po = ps.tile([56, 2, 222], F32, tag="ps")
                nc.tensor.matmul(po[:], tpa[:], zm[(0, j)][:], start=True, stop=False)
                nc.tensor.matmul(po[:], tpb[:], zm[(1, j)][:], start=False, stop=True)
                os_ = sb.tile([56, 2, 222], BF16, tag=f"os{j}")
                nc.any.tensor_copy(os_[:], po[:])
                o_sb.append(os_)

            # S12: transpose -> oT [ls, 4c, 56]
            oT = []
            for s in range(2):
                ls = 112 if s == 0 else 110
                pot = ps.tile([ls, G, 56], BF16, tag="ps")
                for j in range(2):
                    for p in range(2):
                        nc.tensor.transpose(
                            pot[:, 2 * j + p, :],
                            o_sb[j][:, p, 112 * s : 112 * s + ls],
                            ident[0:56, 0:56],
                        )
                ot = sb.tile([ls, G, 56], BF16, tag=f"ot{s}")
                nc.any.tensor_copy(ot[:], pot[:])
                oT.append(ot)

            # S14: post W-conv -> pf [56 W''', 4c, 56 H''']
            pf = ps.tile([56, G, 56], F32, tag="ps")
            nc.tensor.matmul(pf[:], tpa[:], oT[0][:], start=True, stop=False)
            nc.tensor.matmul(pf[:], tpb[:], oT[1][:], start=False, stop=True)
            fsb = sb.tile([56, G, 56], BF16, tag="fsb")
            nc.any.tensor_copy(fsb[:], pf[:])

            # S16: final transpose -> [56 H''', 4c, 56 W''']
            pft = ps.tile([56, G, 56], BF16, tag="ps")
            for p in range(G):
                nc.tensor.transpose(pft[:, p, :], fsb[:, p, :], ident[0:56, 0:56])
            osb = sb.tile([56, G, 56], F32, tag="osb")
            nc.any.tensor_copy(osb[:], pft[:])
            nc.sync.dma_start(out[n, c0 : c0 + G, :, :].rearrange("c h w -> h c w"), osb[:])

    if split:
        split_multi_waits(nc)
    return nc


_CACHE = {}


def kernel(inputs, lpf_pre, lpf_post):
    inputs = np.asarray(inputs)
    tpre, tpost = _build_toeplitz(np.asarray(lpf_pre), np.asarray(lpf_post))
    bf = ml_dtypes.bfloat16
    consts = {
        "tpre": tpre.astype(bf),
        "tpost_a": tpost[:112].astype(bf),
        "tpost_b": tpost[112:].astype(bf),
        "identb": np.eye(128, dtype=np.float32).astype(bf),
    }
    if "nc" not in _CACHE:
        _CACHE["nc"] = _build()
    nc = _CACHE["nc"]
    in_maps = [
        {"x": np.ascontiguousarray(inputs[N_PER * i : N_PER * (i + 1)]), **consts}
        for i in range(N_CORES)
    ]
    global _LAST_IN_MAPS
    _LAST_IN_MAPS = in_maps
    res = run_bass_kernel_spmd(nc, in_maps, core_ids=list(range(N_CORES)))
    return np.concatenate([res.results[i]["out"] for i in range(N_CORES)], axis=0)

